# revision 114
# baseline (speedup 1.0000x reference)
"""Fused AttentionBlock (GroupNorm + single-head attention + proj + residual)
for Trainium2, Bass/Tile, data-parallel over batch across 8 NeuronCores.

Math (per sample, C=256 channels, N=1024 spatial):
  xn = GroupNorm(x) * gn_w + gn_b            (8 groups of 32 channels)
  u  = (Wk^T Wq)^T @ xn                      (fused q.k operand, fp8)
  S^T[k,q] = sum_c u[c,k] xn[c,q]            (fp8 DoubleRow matmul, K=256)
  P = exp(S^T * scale + kb[k])               (kb = k-side logit bias; the
                                              q-side bias drops under softmax)
  O_un[c,q] = sum_k v[c,k] P[k,q]            (fp8 DoubleRow over kt pairs)
  out = proj_w @ (O_un / (4 colsum P)) + proj_b' + x
                                             (v carries a host-side x4 so
                                              osb = po * recip exactly;
                                              proj_b' folds the v bias)

Key design points vs the fp32r version (99.6us -> 72.4us):
- Every attention matmul runs in fp8e4 with MatmulPerfMode.DoubleRow:
  0.5 cycles/row and the full K=256 contraction in ONE instruction (the
  [128, 2, N] operand layout packs both channel/kt halves; DR LDWEIGHTS
  requires the pair-dim stride to be a multiple of 128 elements, hence
  the padded weight tensor).  PE drops ~3.5x.  x is stored bf16
  host-side (halves DMA + residual reads).  rel_err ~1.8e-3 (gate 2e-2).
- exp is the ACT bottleneck (1 elem/lane/cycle, 1M elems/sample): the S
  matmuls for BOTH q-halves of a k-tile land in one 2-bank PSUM tile so
  each exp is a single [128, 1024] op with the per-partition k-bias port
  intact -- 8 ACT ops per sample instead of 16.
- The k-bias comes from dedicated tiny DoubleRow matmuls into a [128, 8]
  PSUM tile (one DVE copy out), not from per-kt PSUM-column copies.
- Denominators accumulate in PSUM via 4.0-constant DoubleRow matmuls over
  the same fp8 P tiles the O-matmul streams (no DVE merge tree).
- Hardware rules the cost model does not check: GPSIMD can neither read
  PSUM nor run scalar-ptr ops, the PE has no PSUM read port, and DVE
  has no divide -- so ALL PSUM drains (u, v, O, denominators, proj
  outputs) split across DVE and ACT, the proj bias rides a K=1
  accumulating matmul (bpr row x ones row), and the h0 epilogue is ONE
  fused DVE scalar_tensor_tensor (PSUM drain + residual add).
- Deferred O bursts: P tiles for the whole sample live in SBUF (PT).
  h0's burst is emitted in its OWN window at kt7 (pairs 0-2 fill the
  PE's idle slots from exp5 on; only pair 3 waits for exp7, so the next
  sample's S matmuls are not queued behind it); h1's burst runs in the
  next window.  The po/psc trio occupies psB for ~2us instead of a
  whole window; the last sample's h1 trio lives in freed psS banks so
  both drain epilogues overlap.
- PSUM (8 banks): S-pairs 2x2, po/psc/proj/u-staging 3 rotating, v pairs
  + groupnorm + k-bias tiny tiles 1.
"""

import os
from contextlib import ExitStack

import numpy as np

import concourse.bass as bass
import concourse.mybir as mybir
import concourse.tile as tile
from concourse.bass_utils import run_bass_kernel_spmd

# Problem shapes (hardcoded per spec nn_AttentionBlock_62397284876438)
B, C, HIMG, WIMG = 32, 256, 32, 32
HW = HIMG * WIMG          # 1024 spatial positions
G = 8                     # groupnorm groups
EPS = 1e-5
NCORES = 8
NS = B // NCORES          # samples per core = 4
P = 128                   # SBUF partitions
CT = C // P               # channel tiles = 2
NT = HW // P              # spatial tiles = 8
FD = 512                  # matmul moving free dim (one PSUM bank of fp32)
NH = HW // FD             # q halves = 2
SCALE = C ** -0.5
KU = 8.0                  # host scale on wu for fp8 range
KV = 4.0                  # host scale on wv; makes osb = po * (1/(4 colsum))
F32 = mybir.dt.float32
F8 = mybir.dt.float8e4
BF16 = mybir.dt.bfloat16
MM_DT = mybir.dt.float32r
DR = mybir.MatmulPerfMode.DoubleRow

last_results = None       # BassKernelResults of the most recent run (for test.py)
_nc_cache = {}


def _hs(h):
    return slice(h * FD, (h + 1) * FD)


def _ms(m):
    return slice(m * P, (m + 1) * P)


def _build_nc():
    nc = bass.Bass()

    x_d = nc.dram_tensor("x", [NS, CT, P, HW], BF16, kind="ExternalInput")
    W8 = 2 * C + P  # wu | wv | kvec | pad (DR ldweights needs 128-mult stride)
    w8_d = nc.dram_tensor("w8", [CT, P, W8], F8, kind="ExternalInput")
    wb_d = nc.dram_tensor("wb", [CT, P, C], BF16, kind="ExternalInput")
    sm_d = nc.dram_tensor("sm", [P, 6 + CT * G], F32, kind="ExternalInput")
    bcmask_d = nc.dram_tensor("bcmask", [G, CT * P], F32, kind="ExternalInput")
    bpr_d = nc.dram_tensor("bpr", [1, C], BF16, kind="ExternalInput")
    out_d = nc.dram_tensor("out", [NS, CT, P, HW], F32, kind="ExternalOutput")

    AL = mybir.AluOpType
    AF = mybir.ActivationFunctionType

    with tile.TileContext(nc) as tc, ExitStack() as ctx:
        consts = ctx.enter_context(tc.tile_pool(name="consts", bufs=1))
        xpool = ctx.enter_context(tc.tile_pool(name="xpool", bufs=4))
        xnpool = ctx.enter_context(tc.tile_pool(name="xnpool", bufs=2))
        gpool = ctx.enter_context(tc.tile_pool(name="gpool", bufs=2))
        qkpool = ctx.enter_context(tc.tile_pool(name="qkpool", bufs=2))
        vpool = ctx.enter_context(tc.tile_pool(name="vpool", bufs=2))
        kbpool = ctx.enter_context(tc.tile_pool(name="kbpool", bufs=2))
        ptpool = ctx.enter_context(tc.tile_pool(name="ptpool", bufs=2))
        rcpool = ctx.enter_context(tc.tile_pool(name="rcpool", bufs=2))
        opool = ctx.enter_context(tc.tile_pool(name="opool", bufs=3))
        # PSUM: psS 2x [P,2,FD] f32 (4 banks; S-pairs + u staging),
        # psB 3x [P,FD] (po0/po1/psc/proj rotation),
        # psR 1x (1 bank; v, groupnorm, k-bias tiny tiles)
        psS = ctx.enter_context(tc.tile_pool(name="psS", bufs=2, space="PSUM"))
        psB = ctx.enter_context(tc.tile_pool(name="psB", bufs=3, space="PSUM"))
        psR = ctx.enter_context(tc.tile_pool(name="psR", bufs=1, space="PSUM"))

        # ---- constants (x of sample 0 first: groupnorm is the startup
        # critical path) ----
        x0 = xpool.tile([P, CT, HW], BF16, name="x_0", tag="x")
        for hh in range(NH):
            nc.sync.dma_start(out=x0[:, 0, _hs(hh)], in_=x_d[0, 0][:, _hs(hh)])
        nc.sync.dma_start(out=x0[:, 1], in_=x_d[0, 1])
        sm = consts.tile([P, 6 + CT * G], F32, name="sm_sb", tag="sm_sb")
        nc.sync.dma_start(out=sm, in_=sm_d[:])
        bcmask = consts.tile([G, CT * P], F32, name="bcmask_sb", tag="bcmask_sb")
        nc.sync.dma_start(out=bcmask, in_=bcmask_d[:])
        w8 = consts.tile([P, CT, W8], F8, name="w8_sb", tag="w8_sb")
        nc.sync.dma_start(out=w8, in_=w8_d.rearrange("ct p f -> p ct f"))
        # x1 before wb: sample 1's stats data lands early; wb (proj
        # weights) is not needed until the first h-epilogue
        x1 = xpool.tile([P, CT, HW], BF16, name="x_1", tag="x")
        if NS > 1:
            nc.sync.dma_start(out=x1[:, 0], in_=x_d[1, 0])
        wb = consts.tile([P, CT, C], BF16, name="wb_sb", tag="wb_sb")
        bpr = consts.tile([1, C], BF16, name="bpr_sb", tag="bpr_sb")
        nc.sync.dma_start(out=bpr, in_=bpr_d[:])
        onesrow = consts.tile([1, FD], BF16, name="onesrow_sb", tag="onesrow_sb")
        nc.vector.memset(onesrow, 1.0)
        bp = sm[:, 0:CT]
        gnw = sm[:, CT:2 * CT]
        gnb = sm[:, 2 * CT:3 * CT]
        # 4.0-constant lhsT for the colsum (denominator) DoubleRow matmuls
        fours = consts.tile([P, 2, P], F8, name="fours_sb", tag="fours_sb")
        nc.vector.memset(fours, 4.0)
        epst = consts.tile([P, 1], F32, name="eps_sb", tag="eps_sb")
        nc.vector.memset(epst, EPS)
        # PE warmup rides the fours constant (no dedicated f32r tiles or
        # DVE memsets delaying sample 0's bn_stats in the startup queue)
        wps = psB.tile([P, P], F32, name="warm_ps", tag="po")
        nc.tensor.matmul(wps, lhsT=fours[:, 0], rhs=fours[:, 0],
                         start=True, stop=True, skip_group_check=True)

        def emit_load(s):
            if s == 0:
                return x0
            if s == 1:
                return x1
            xs = xpool.tile([P, CT, HW], BF16, name=f"x_{s}", tag="x")
            nc.sync.dma_start(out=xs, in_=x_d[s].rearrange("ct p f -> p ct f"))
            return xs

        def emit_gn_stats1(s, xs, only_ct=None):
            # per-channel stats (DVE only); emitted per-ct so a pending
            # sample's stats can't head-of-line-block the critical affine
            if only_ct in (None, 0):
                st6 = gpool.tile([P, CT, 2, 6], BF16, name=f"st6_{s}",
                                 tag="st6")
                mv = gpool.tile([P, CT, 2], BF16, name=f"mv_{s}", tag="mv")
                ms = gpool.tile([P, CT, 2], F32, name=f"ms_{s}", tag="ms")
                emit_gn_stats1.cur = (st6, mv, ms)
            st6, mv, ms = emit_gn_stats1.cur
            cts = range(CT) if only_ct is None else [only_ct]
            for ct in cts:
                for h in range(2):
                    nc.vector.bn_stats(out=st6[:, ct, h], in_=xs[:, ct, _hs(h)])
                nc.vector.bn_aggr(out=mv[:, ct], in_=st6[:, ct])
                # ms = [mean, E[x^2]] per channel; E[x^2] = mean^2 + var
                nc.vector.tensor_copy(ms[:, ct, 0:1], mv[:, ct, 0:1])
                nc.vector.scalar_tensor_tensor(
                    out=ms[:, ct, 1:2], in0=mv[:, ct, 0:1],
                    scalar=mv[:, ct, 0:1], in1=mv[:, ct, 1:2],
                    op0=AL.mult, op1=AL.add)
            return ms

        def emit_gn_stats2(s, ms):
            # group aggregate (tiny PE matmul) + rstd chain
            gps = psR.tile([G, 2], F32, name=f"gps_{s}", tag="ps")
            for ct in range(CT):
                nc.tensor.matmul(gps,
                                 lhsT=sm[:, 3 * CT + ct * G:3 * CT + (ct + 1) * G],
                                 rhs=ms[:, ct],
                                 start=(ct == 0), stop=(ct == CT - 1))
            graw = gpool.tile([G, 2], F32, name=f"graw_{s}", tag="graw")
            gtmp = gpool.tile([G, 2], F32, name=f"gtmp_{s}", tag="gtmp")
            grs = gpool.tile([G, 2], F32, name=f"grs_{s}", tag="grs")
            nc.vector.tensor_copy(graw, gps)
            nc.vector.tensor_tensor(out=gtmp[:, 0:1], in0=graw[:, 0:1],
                                    in1=graw[:, 0:1], op=AL.mult)
            nc.vector.tensor_tensor(out=gtmp[:, 1:2], in0=graw[:, 1:2],
                                    in1=gtmp[:, 0:1], op=AL.subtract)
            # rstd = exp(-0.5*ln(var+eps)): same ACT table set as softmax exp
            nc.scalar.activation(out=gtmp[:, 0:1], in_=gtmp[:, 1:2],
                                 func=AF.Ln, bias=epst[:G])
            nc.scalar.activation(out=grs[:, 1:2], in_=gtmp[:, 0:1],
                                 func=AF.Exp, scale=-0.5)
            nc.vector.tensor_copy(grs[:, 0:1], graw[:, 0:1])
            return grs

        def emit_gn_affine(s, grs, xs):
            # broadcast per-group [mean, rstd] back to channels, build affine,
            # write xn directly in fp8 (this is also the bf16->fp8 cast)
            AB = gpool.tile([P, CT, 2], F32, name=f"AB_{s}", tag="AB")
            xn = xnpool.tile([P, CT, HW], F8, name=f"xn_{s}", tag="xn")
            # both ct broadcasts into ONE psum tile: single alloc/release, no
            # psR rotation stall between ct0 and ct1
            bc = psR.tile([P, CT, 2], F32, name=f"bc_{s}", tag="ps")
            for ct in range(CT):
                nc.tensor.matmul(bc[:, ct], lhsT=bcmask[:, ct * P:(ct + 1) * P],
                                 rhs=grs, start=True, stop=True,
                                 skip_group_check=True)
            for ct in range(CT):
                nc.vector.tensor_tensor(out=AB[:, ct, 0:1], in0=bc[:, ct, 1:2],
                                        in1=gnw[:, ct:ct + 1], op=AL.mult)
                nc.vector.tensor_tensor(out=AB[:, ct, 1:2], in0=bc[:, ct, 0:1],
                                        in1=AB[:, ct, 0:1], op=AL.mult)
                nc.vector.tensor_tensor(out=AB[:, ct, 1:2], in0=gnb[:, ct:ct + 1],
                                        in1=AB[:, ct, 1:2], op=AL.subtract)
                for hh in range(NH):
                    nc.vector.tensor_scalar(
                        out=xn[:, ct, _hs(hh)], in0=xs[:, ct, _hs(hh)],
                        scalar1=AB[:, ct, 0:1], scalar2=AB[:, ct, 1:2],
                        op0=AL.mult, op1=AL.add)
            return xn

        def emit_kb(s, xn):
            # k-side logit bias for all 8 k-tiles: tiny DoubleRow matmuls
            # into one [128, 8] PSUM tile, one DVE copy out
            kbps = psR.tile([P, NT], F32, name=f"kbps_{s}", tag="ps")
            for kt in range(NT):
                nc.tensor.matmul(
                    kbps[:, kt:kt + 1],
                    lhsT=xn[:, :, _ms(kt)],
                    rhs=w8[:, :, 2 * C:2 * C + 1],
                    start=True, stop=True, perf_mode=DR,
                    skip_group_check=True)
            kbt = kbpool.tile([P, NT], F32, name=f"kbt_{s}", tag="kbt")
            nc.vector.tensor_copy(kbt, kbps)
            return kbt

        def emit_u_half(s, xn, u, h):
            # one q-half of u = (Wk^T Wq)^T @ xn: two DoubleRow matmuls into
            # single-bank psB tiles (psB idles mid-window now that the O
            # bursts cycle it quickly).  GPSIMD cannot read PSUM, so the
            # copies split DVE/ACT.
            for m in range(CT):
                ps = psB.tile([P, FD], F32, name=f"u_ps_{s}_{h}_{m}",
                              tag="po")
                nc.tensor.matmul(
                    ps,
                    lhsT=w8[:, :, _ms(m)],
                    rhs=xn[:, :, _hs(h)],
                    start=True, stop=True, perf_mode=DR)
                if m == 0:
                    nc.vector.tensor_copy(u[:, m, _hs(h)], ps)
                else:
                    nc.scalar.copy(out=u[:, m, _hs(h)], in_=ps)

        def emit_v_pair(s, xn, vT, j):
            # two spatial tiles of v (transposed, fp8, x4 host scale) into
            # one full PSUM bank, one copy out (DVE/ACT alternating; GPSIMD
            # cannot read PSUM)
            ps = psR.tile([P, 2, C], F32, name=f"v_ps_{s}_{j}", tag="ps")
            for i in range(2):
                nc.tensor.matmul(
                    ps[:, i],
                    lhsT=xn[:, :, _ms(2 * j + i)],
                    rhs=w8[:, :, C:2 * C],
                    start=True, stop=True, perf_mode=DR,
                    skip_group_check=True)
            nc.vector.tensor_copy(vT[:, 2 * j:2 * j + 2], ps)

        def emit_skt(s, kt, xn, u, PT, kbt, split=False):
            # S^T for both q-halves of k-tile kt into one 2-bank PSUM pair,
            # then a single [128,1024] exp with the per-partition k-bias.
            # split=True (drain tail) runs two half exps h0-first so the h0
            # epilogue chain starts before the stream fully ends.
            sp = psS.tile([P, 2, FD], F32, name=f"sp_{s}_{kt}", tag="sp")
            for h in range(NH):
                nc.tensor.matmul(
                    sp[:, h],
                    lhsT=u[:, :, _ms(kt)],
                    rhs=xn[:, :, _hs(h)],
                    start=True, stop=True, perf_mode=DR)
            if split:
                for h in range(NH):
                    nc.scalar.activation(out=PT[:, kt, h], in_=sp[:, h],
                                         func=AF.Exp, scale=SCALE / KU,
                                         bias=kbt[:, kt:kt + 1])
            else:
                nc.scalar.activation(out=PT[:, kt], in_=sp,
                                     func=AF.Exp, scale=SCALE / KU,
                                     bias=kbt[:, kt:kt + 1])

        def emit_oj(s, h, j, vT, PT, po, psc):
            # O and colsum accumulation for kt pair j of half h
            rhs = PT[:, 2 * j:2 * j + 2, h, :]
            for m in range(CT):
                nc.tensor.matmul(
                    po[m],
                    lhsT=vT[:, 2 * j:2 * j + 2, _ms(m)],
                    rhs=rhs,
                    start=(j == 0), stop=(j == NT // 2 - 1), perf_mode=DR)
            nc.tensor.matmul(
                psc,
                lhsT=fours,
                rhs=rhs,
                start=(j == 0), stop=(j == NT // 2 - 1), perf_mode=DR)

        def emit_hfinish(s, h, xs, po, psc, projpool=None, drain=False):
            # denominator reciprocal, normalize, bf16 proj; the PSUM drain
            # (+bias via the ACT/DVE port) and the SBUF residual add split
            # across engines -- GPSIMD only ever touches SBUF
            projpool = projpool or psB
            osb = opool.tile([P, CT, FD], BF16, name=f"osb_{s}_{h}", tag="osb")
            recip = rcpool.tile([P, FD], F32, name=f"recip_{s}_{h}",
                                tag="recip")
            nc.vector.reciprocal(out=recip, in_=psc)
            for m in range(CT):
                nc.vector.tensor_tensor(out=osb[:, m], in0=po[m],
                                        in1=recip, op=AL.mult)
            outp = opool.tile([P, CT, FD], F32, name=f"outp_{s}_{h}",
                              tag="outp")
            for m in range(CT):
                ps = projpool.tile([P, FD], F32, name=f"p_ps_{s}_{h}_{m}",
                                   tag="po" if projpool is psB else "sp")
                for ct in range(CT):
                    nc.tensor.matmul(
                        ps,
                        lhsT=wb[:, ct, _ms(m)],
                        rhs=osb[:, ct],
                        start=(ct == 0), stop=False)
                # proj bias rides a K=1 accumulating matmul (PE is cheap;
                # Pool supports neither PSUM reads nor scalar-ptr ops)
                nc.tensor.matmul(ps, lhsT=bpr[:, _ms(m)], rhs=onesrow,
                                 start=False, stop=True)
                # h0: ONE fused DVE stt does PSUM drain + residual add
                # (bias is already accumulated in PSUM); h1: ACT drains,
                # Pool adds the residual in SBUF.
                if (h == 0 and not drain) or (drain and h == 1):
                    nc.vector.scalar_tensor_tensor(
                        out=outp[:, m], in0=ps, scalar=0.0,
                        in1=xs[:, m, _hs(h)], op0=AL.add, op1=AL.add)
                else:
                    nc.scalar.copy(out=outp[:, m], in_=ps)
                    radd = nc.vector if (drain and h == 1 and m == 1) \
                        else nc.gpsimd
                    radd.tensor_tensor(out=outp[:, m], in0=outp[:, m],
                                       in1=xs[:, m, _hs(h)], op=AL.add)
                nc.sync.dma_start(out=out_d[s, m][:, _hs(h)],
                                  in_=outp[:, m])

        # ---- prologue: sample 0's load/GN/u/kb before the tick pipeline ----
        xs_l = [None] * NS
        xn_l = [None] * NS
        u_l = [None] * NS
        kbt_l = [None] * NS
        xs_l[0] = emit_load(0)
        with tc.high_priority():
            ms0 = emit_gn_stats1(0, xs_l[0])
            grs0 = emit_gn_stats2(0, ms0)
            xn_l[0] = emit_gn_affine(0, grs0, xs_l[0])
            kbt_l[0] = emit_kb(0, xn_l[0])
            u_l[0] = qkpool.tile([P, CT, HW], F8, name="u_0", tag="u")
            emit_u_half(0, xn_l[0], u_l[0], 0)
            emit_u_half(0, xn_l[0], u_l[0], 1)
        if NS > 1:
            xs_l[1] = emit_load(1)
            nc.sync.dma_start(out=x1[:, 1], in_=x_d[1, 1])
        nc.sync.dma_start(out=wb, in_=wb_d.rearrange("ct p f -> p ct f"))

        # ---- flat tick pipeline ----
        vT_l = [None] * NS
        PT_l = [None] * NS
        po0_l = [None] * NS   # h0 trio (allocated at kt5)
        po1_l = [None] * NS   # h1 trio (allocated in next sample's window)
        ms_nxt = None
        for t in range(NS * NT):
            s, kt = divmod(t, NT)
            if kt == 0:
                vT_l[s] = vpool.tile([P, NT, C], F8, name=f"vT_{s}", tag="vT")
                PT_l[s] = ptpool.tile([P, NT, 2, FD], F8, name=f"PT_{s}",
                                      tag="PT")
            # S + exp first (exp paces everything), then v
            emit_skt(s, kt, xn_l[s], u_l[s], PT_l[s], kbt_l[s],
                     split=(s == NS - 1 and kt == 7))
            if kt % 2 == 0:
                emit_v_pair(s, xn_l[s], vT_l[s], kt // 2)
            # previous sample's O bursts + epilogues: all of PT is in
            # SBUF, so each h's O/cs runs back-to-back and the po/psc trio
            # only occupies psB for ~2us instead of a whole sample window
            if s > 0:
                sp_ = s - 1
                if kt == 0:
                    emit_hfinish(sp_, 0, xs_l[sp_], po0_l[sp_][0],
                                 po0_l[sp_][1])
                if kt == 0:
                    trio = (
                        [psB.tile([P, FD], F32, name=f"po_{sp_}_1_{m}",
                                  tag="po") for m in range(CT)],
                        psB.tile([P, FD], F32, name=f"psc_{sp_}_1",
                                 tag="po"))
                    for j in range(NT // 2):
                        emit_oj(sp_, 1, j, vT_l[sp_], PT_l[sp_],
                                trio[0], trio[1])
                    emit_hfinish(sp_, 1, xs_l[sp_], trio[0], trio[1])
            # h0's O burst in the SAME window (kt7): pairs 0-2 are ready
            # after exp5 and fill the PE's idle slots during the stream;
            # only pair 3 waits for exp7, so the next sample's S matmuls
            # are not queued behind the whole burst
            if kt == 7:
                po0_l[s] = (
                    [psB.tile([P, FD], F32, name=f"po_{s}_0_{m}", tag="po")
                     for m in range(CT)],
                    psB.tile([P, FD], F32, name=f"psc_{s}_0", tag="po"))
                for j in range(NT // 2):
                    emit_oj(s, 0, j, vT_l[s], PT_l[s],
                            po0_l[s][0], po0_l[s][1])
            # next sample's pipeline stages (early: u copies must clear the
            # psS rotation before the next sample's S-pairs need it, and the
            # gn ACT/DVE tinies must not land at the end of this sample's
            # exp stream)
            if kt == 0 and s + 2 < NS:
                xs_l[s + 2] = emit_load(s + 2)
            if kt == 0 and s + 1 < NS:
                ms_nxt = emit_gn_stats1(s + 1, xs_l[s + 1])
            if kt == 1 and s + 1 < NS:
                grs_nxt = emit_gn_stats2(s + 1, ms_nxt)
            if kt == 3 and s + 1 < NS:
                xn_l[s + 1] = emit_gn_affine(s + 1, grs_nxt, xs_l[s + 1])
            if kt == 4 and s + 1 < NS:
                kbt_l[s + 1] = emit_kb(s + 1, xn_l[s + 1])
                u_l[s + 1] = qkpool.tile([P, CT, HW], F8,
                                         name=f"u_{s + 1}", tag="u")
                emit_u_half(s + 1, xn_l[s + 1], u_l[s + 1], 0)
                emit_u_half(s + 1, xn_l[s + 1], u_l[s + 1], 1)

        # ---- drain: last sample's epilogues, h0/h1 chains overlapped ----
        # h1's trio lives in psS (free once the last exps retire) so its
        # O/cs run concurrently with h0's psB-based finish chain
        sl = NS - 1
        poA = psS.tile([P, 2, FD], F32, name="po_l1", tag="sp")
        pscl = psS.tile([P, FD], F32, name="psc_l1", tag="sp")
        po1l = [poA[:, 0], poA[:, 1]]
        for j in range(NT // 2):
            emit_oj(sl, 1, j, vT_l[sl], PT_l[sl], po1l, pscl)
        emit_hfinish(sl, 0, xs_l[sl], po0_l[sl][0], po0_l[sl][1], drain=True)
        emit_hfinish(sl, 1, xs_l[sl], po1l, pscl, projpool=psS, drain=True)

    import bass_rust
    bass_rust.generate_event_semaphores(nc)
    return nc


def _get_nc():
    if "nc" not in _nc_cache:
        _nc_cache["nc"] = _build_nc()
    return _nc_cache["nc"]


def _prep_consts(gn_w, gn_b, qkv_w, qkv_b, proj_w, proj_b):
    f = np.float32
    c = np.ascontiguousarray
    import ml_dtypes
    f8 = ml_dtypes.float8_e4m3
    Wq = qkv_w[:C].astype(np.float64)
    Wk = qkv_w[C:2 * C].astype(np.float64)
    bq = qkv_b[:C].astype(np.float64)
    scale = C ** -0.5
    wu = (Wk.T @ Wq) * KU                     # kernel matmul transposes
    kvec = scale * (Wk.T @ bq)                # k-side logit bias vector
    wv = qkv_w[2 * C:].T.astype(np.float64) * KV
    w8 = c(np.concatenate([wu, wv, kvec[:, None], np.zeros((C, P - 1))], axis=1)
           .reshape(CT, P, 2 * C + P).astype(f)).astype(f8)
    wb = c(proj_w.T.astype(f).reshape(CT, P, C)).astype(ml_dtypes.bfloat16)
    # softmax rows sum to 1: fold proj_w @ v_bias into the proj bias
    bp_eff = (proj_b.astype(np.float64)
              + proj_w.astype(np.float64) @ qkv_b[2 * C:].astype(np.float64))
    _prep_consts.bp_eff = bp_eff.astype(f)
    bp = bp_eff.astype(f).reshape(CT, P).T
    gnw = gn_w.reshape(CT, P).T.astype(f)
    gnb = gn_b.reshape(CT, P).T.astype(f)
    cidx = np.arange(C)
    grp = cidx // (C // G)
    gmask = np.zeros((CT, P, G), f)
    gmask[cidx // P, cidx % P, grp] = 1.0 / (C // G)
    sm = c(np.concatenate(
        [bp, gnw, gnb, gmask.transpose(1, 0, 2).reshape(P, CT * G)], axis=1))
    bcmask = np.zeros((G, CT * P), f)
    bcmask[grp, cidx] = 1.0
    import ml_dtypes as mld
    bpr = c(bp_eff.astype(f).reshape(1, C)).astype(mld.bfloat16)
    return dict(w8=w8, wb=wb, sm=sm, bcmask=bcmask, bpr=bpr)


def kernel(x, gn_w, gn_b, qkv_w, qkv_b, proj_w, proj_b):
    global last_results
    import ml_dtypes
    xf = np.asarray(x, dtype=np.float32)
    x = np.ascontiguousarray(xf.astype(ml_dtypes.bfloat16))
    consts = _prep_consts(
        np.asarray(gn_w, np.float32), np.asarray(gn_b, np.float32),
        np.asarray(qkv_w, np.float32), np.asarray(qkv_b, np.float32),
        np.asarray(proj_w, np.float32), np.asarray(proj_b, np.float32))
    nc = _get_nc()
    xr = x.reshape(NCORES, NS, CT, P, HW)
    in_maps = [dict(x=np.ascontiguousarray(xr[i]), **consts)
               for i in range(NCORES)]
    trace = bool(int(os.environ.get("ATTN_TRACE", "0")))
    last_results = run_bass_kernel_spmd(
        nc, in_maps, core_ids=list(range(NCORES)), trace=trace)
    out = np.stack([r["out"] for r in last_results.results])
    return out.reshape(B, C, HIMG, WIMG)
